# revision 28
# baseline (speedup 1.0000x reference)
"""Bass/Trainium2 kernel for nn_GroundingLoss (symmetric token-level InfoNCE).

Math (matches the jax reference exactly):
    sim[a,b,i,j] = sum_k x[a,i,k] * z[b,j,k]
    S[a,b]       = (1/J) * sum_j  [ sum_i softmax_i(sim[a,b,:,j]) * sim[a,b,:,j] ]
    loss         = mean( logsumexp_a(S) - diag + logsumexp_b(S) - diag )

Sharding: the batch axis of x (a) is split across the 8 cores; every core
computes S[a_local, :] against all of z.

Device layout per core (v7): partitions = (b4, j32) per (b,j)-tile (64 tiles
of 128), free = (i, a) with i major, so the softmax i-reduction sits on the
FREE axis and the PE only does the single sim pass.  Single-tile pipeline
stages with PSUM bufs=4 — the PE->ACT->DVE chain needs depth >= 3 or the PE
stalls waiting for PSUM (measured 6us stalls at depth 2).  Per tile:
  PE   4 matmuls [128,512] (K=256 as 2 accum halves, weights reused)
  ACT  e  = exp(sim - SHIFT)     (sole exp engine)
  DVE  es = e * sim              (sole PSUM-capable vector engine; 1024
       elems at ~1.3ns/elem makes this the ~88us floor of the kernel)
  Pool l1e = e i-half fold       (Pool is slow, ~2.5ns/elem, SBUF-only;
       this 512-elem add is all it can afford)
es ships RAW (bf16, 2KB/part/tile) and e ships half-folded (1KB/part/tile),
both on the otherwise-idle SP hwdge queue (~76us, under the span; 28MB HBM
out of ~330GB/s).  Input loads ride the ACT hwdge queue up front.  The host
does the remaining folds in fp32, divides num/den, averages over j, and
runs the tiny [256,256] logsumexp epilogue (softmax weights are
shift-invariant, so no SHIFT correction is needed).
"""

import numpy as np

N, I, J, K = 256, 32, 32, 256
NCORES = 8
NL = N // NCORES          # 32 local a's per core
AF = NL * I               # 1024 rhs cols per K-half (i, a) i-major
BJ = N * J                # 8192 (b, j) pairs
NT = BJ // 128            # 64 (b,j)-tiles of 128 partitions
SHIFT = 60.0              # exp shift: safe for |sim| up to ~130

_cached = None


def _build():
    import concourse.bacc as bacc
    import concourse.mybir as mybir
    import concourse.tile as tile

    f32 = mybir.dt.float32
    bf16 = mybir.dt.bfloat16
    AF_T = mybir.ActivationFunctionType

    nc = bacc.Bacc("TRN2", target_bir_lowering=False, debug=False)
    xt_d = nc.dram_tensor("xt", [128, 2 * AF], bf16, kind="ExternalInput").ap()
    zt_d = nc.dram_tensor("zt", [128, 2 * BJ], bf16, kind="ExternalInput").ap()
    os_d = nc.dram_tensor("os", [128, NT, 1536], bf16, kind="ExternalOutput").ap()

    with tile.TileContext(nc) as tc:
        with (
            tc.tile_pool(name="const", bufs=1) as cpool,
            tc.tile_pool(name="psum", bufs=4, space="PSUM") as ppool,
            tc.tile_pool(name="ees", bufs=6) as epool,
        ):
            bias_t = cpool.tile([128, 1], f32)
            nc.gpsimd.memset(bias_t[:], -SHIFT)
            xt = cpool.tile([128, 2 * AF], bf16)
            # zt as separate per-chunk tiles so the first matmuls only wait
            # on their own chunk (one [128, CW] region is one dependency
            # unit); loads split across the two hwdge queues
            nq = 8
            CW = BJ // nq  # 1024 cols = 8 tiles per chunk
            nc.sync.dma_start(xt[:, 0:AF], xt_d[:, 0:AF])
            nc.scalar.dma_start(xt[:, AF : 2 * AF], xt_d[:, AF : 2 * AF])
            zq = [[None] * nq for _ in range(2)]
            for q in range(nq):
                for kc in range(2):
                    zq[kc][q] = cpool.tile([128, CW], bf16, name=f"zq{kc}_{q}")
                    eng = nc.sync if kc == 0 else nc.scalar
                    eng.dma_start(zq[kc][q][:], zt_d[:, kc * BJ + q * CW : kc * BJ + (q + 1) * CW])

            for t in range(NT):
                sim = ppool.tile([128, 1024], f32, tag="sim")  # (i, a) flat
                for kc in range(2):
                    zch = zq[kc][t // 8]
                    lhsT = zch[:, (t % 8) * 128 : (t % 8 + 1) * 128]
                    for ih in range(2):
                        nc.tensor.matmul(
                            sim[:, ih * 512 : (ih + 1) * 512],
                            lhsT,
                            xt[:, kc * AF + ih * 512 : kc * AF + (ih + 1) * 512],
                            start=(kc == 0),
                            stop=(kc == 1),
                        )
                # [e 1024 | es 1024 | l1e 512]: one contiguous out-DMA per tile
                ees = epool.tile([128, 2560], bf16, tag="ees")
                nc.scalar.activation(ees[:, 0:1024], sim[:], AF_T.Exp, bias=bias_t[:], scale=1.0)
                nc.vector.tensor_mul(ees[:, 1024:2048], ees[:, 0:1024], sim[:])
                nc.gpsimd.tensor_add(ees[:, 2048:2560], ees[:, 0:512], ees[:, 512:1024])
                nc.sync.dma_start(os_d[:, t], ees[:, 1024:2560])
    nc.compile()
    return nc


def _prep_inputs(x, z):
    import ml_dtypes

    bf = ml_dtypes.bfloat16
    x = np.ascontiguousarray(x, dtype=np.float32).astype(bf)
    z = np.ascontiguousarray(z, dtype=np.float32).astype(bf)
    # zt[p, kc*BJ + b*J + j] = z[b, j, kc*128 + p]
    zt = z.transpose(2, 0, 1).reshape(K, BJ)
    zt = np.concatenate([zt[0:128], zt[128:256]], axis=1)
    zt = np.ascontiguousarray(zt)
    in_maps = []
    for d in range(NCORES):
        xl = x[d * NL : (d + 1) * NL]                  # [a, i, K]
        xt = xl.transpose(2, 1, 0).reshape(K, AF)      # [K, (i, a)]
        xt = np.concatenate([xt[0:128], xt[128:256]], axis=1)
        in_maps.append({"xt": np.ascontiguousarray(xt), "zt": zt})
    return in_maps


def _epilogue(results):
    S = np.empty((N, N), dtype=np.float64)
    for d in range(NCORES):
        arr = results[d]["os"].astype(np.float32).reshape(128, NT, 1536)
        num = arr[:, :, 0:1024].reshape(128, NT, I, NL).sum(axis=2)    # [(b4,j), t, a]
        den = arr[:, :, 1024:1536].reshape(128, NT, 16, NL).sum(axis=2)
        r = num / den
        r = r.reshape(4, J, NT, NL).mean(axis=1).astype(np.float64)  # [b4, t, a]
        S[d * NL : (d + 1) * NL, :] = r.transpose(2, 1, 0).reshape(NL, N)
    diag = np.diagonal(S)
    m0 = S.max(axis=0)
    lx = m0 + np.log(np.exp(S - m0[None, :]).sum(axis=0)) - diag
    m1 = S.max(axis=1)
    lz = m1 + np.log(np.exp(S - m1[:, None]).sum(axis=1)) - diag
    loss = (lx + lz).mean()
    return np.asarray(loss, dtype=np.float32)


def run_on_device(x, z, trace=False):
    """Returns (loss, BassKernelResults)."""
    from concourse.bass_utils import run_bass_kernel_spmd

    global _cached
    if _cached is None:
        _cached = _build()
    nc = _cached
    in_maps = _prep_inputs(x, z)
    res = run_bass_kernel_spmd(nc, in_maps, list(range(NCORES)), trace=trace)
    return _epilogue(res.results), res


def kernel(x, z):
    loss, _ = run_on_device(x, z)
    return loss


# revision 29
# speedup vs baseline: 1.1896x; 1.1896x over previous
"""Bass/Trainium2 kernel for nn_GroundingLoss (symmetric token-level InfoNCE).

Math (matches the jax reference exactly):
    sim[a,b,i,j] = sum_k x[a,i,k] * z[b,j,k]
    S[a,b]       = (1/J) * sum_j  [ sum_i softmax_i(sim[a,b,:,j]) * sim[a,b,:,j] ]
    loss         = mean( logsumexp_a(S) - diag + logsumexp_b(S) - diag )

Sharding: the batch axis of x (a) is split across the 8 cores; every core
computes S[a_local, :] against all of z.

Device layout per core (v7): partitions = (b4, j32) per (b,j)-tile (64 tiles
of 128), free = (i, a) with i major, so the softmax i-reduction sits on the
FREE axis and the PE only does the single sim pass.  Single-tile pipeline
stages with PSUM bufs=4 — the PE->ACT->DVE chain needs depth >= 3 or the PE
stalls waiting for PSUM (measured 6us stalls at depth 2).  Per tile:
  PE   4 matmuls [128,512] (K=256 as 2 accum halves, weights reused)
  ACT  e  = exp(sim - SHIFT)     (sole exp engine)
  DVE  es = e * sim              (sole PSUM-capable vector engine; 1024
       elems at ~1.3ns/elem makes this the ~88us floor of the kernel)
  Pool l1e = e i-half fold       (Pool is slow, ~2.5ns/elem, SBUF-only;
       this 512-elem add is all it can afford)
es ships RAW (bf16, 2KB/part/tile) and e ships half-folded (1KB/part/tile),
both on the otherwise-idle SP hwdge queue (~76us, under the span; 28MB HBM
out of ~330GB/s).  Input loads ride the ACT hwdge queue up front.  The host
does the remaining folds in fp32, divides num/den, averages over j, and
runs the tiny [256,256] logsumexp epilogue (softmax weights are
shift-invariant, so no SHIFT correction is needed).
"""

import numpy as np

N, I, J, K = 256, 32, 32, 256
NCORES = 8
NL = N // NCORES          # 32 local a's per core
AF = NL * I               # 1024 rhs cols per K-half (i, a) i-major
BJ = N * J                # 8192 (b, j) pairs
NT = BJ // 128            # 64 (b,j)-tiles of 128 partitions
SHIFT = 60.0              # exp shift: safe for |sim| up to ~130

_cached = None


def _build():
    import concourse.bacc as bacc
    import concourse.mybir as mybir
    import concourse.tile as tile

    f32 = mybir.dt.float32
    bf16 = mybir.dt.bfloat16
    AF_T = mybir.ActivationFunctionType

    nc = bacc.Bacc("TRN2", target_bir_lowering=False, debug=False)
    xt_d = nc.dram_tensor("xt", [128, 2 * AF], bf16, kind="ExternalInput").ap()
    zt_d = nc.dram_tensor("zt", [128, 2 * BJ], bf16, kind="ExternalInput").ap()
    os_d = nc.dram_tensor("os", [128, NT, 1536], bf16, kind="ExternalOutput").ap()

    with tile.TileContext(nc) as tc:
        with (
            tc.tile_pool(name="const", bufs=1) as cpool,
            tc.tile_pool(name="psum", bufs=4, space="PSUM") as ppool,
            tc.tile_pool(name="ees", bufs=6) as epool,
        ):
            bias_t = cpool.tile([128, 1], f32)
            nc.gpsimd.memset(bias_t[:], -SHIFT)
            xt = cpool.tile([128, 2 * AF], bf16)
            # zt as separate per-chunk tiles so the first matmuls only wait
            # on their own chunk (one [128, CW] region is one dependency
            # unit); loads split across the two hwdge queues
            nq = 8
            CW = BJ // nq  # 1024 cols = 8 tiles per chunk
            nc.sync.dma_start(xt[:], xt_d[:, :])
            zq = [[None] * nq for _ in range(2)]
            for q in range(nq):
                for kc in range(2):
                    zq[kc][q] = cpool.tile([128, CW], bf16, name=f"zq{kc}_{q}")
                    eng = nc.sync if kc == 0 else nc.scalar
                    eng.dma_start(zq[kc][q][:], zt_d[:, kc * BJ + q * CW : kc * BJ + (q + 1) * CW])

            for t in range(NT):
                sim = ppool.tile([128, 1024], f32, tag="sim")  # (i, a) flat
                for kc in range(2):
                    zch = zq[kc][t // 8]
                    lhsT = zch[:, (t % 8) * 128 : (t % 8 + 1) * 128]
                    for ih in range(2):
                        nc.tensor.matmul(
                            sim[:, ih * 512 : (ih + 1) * 512],
                            lhsT,
                            xt[:, kc * AF + ih * 512 : kc * AF + (ih + 1) * 512],
                            start=(kc == 0),
                            stop=(kc == 1),
                        )
                # [e 1024 | es 1024 | l1e 512]: one contiguous out-DMA per tile
                ees = epool.tile([128, 2560], bf16, tag="ees")
                nc.scalar.activation(ees[:, 0:1024], sim[:], AF_T.Exp, bias=bias_t[:], scale=1.0)
                nc.vector.tensor_mul(ees[:, 1024:2048], ees[:, 0:1024], sim[:])
                nc.gpsimd.tensor_add(ees[:, 2048:2560], ees[:, 0:512], ees[:, 512:1024])
                nc.sync.dma_start(os_d[:, t], ees[:, 1024:2560])
    nc.compile()
    return nc


def _prep_inputs(x, z):
    import ml_dtypes

    bf = ml_dtypes.bfloat16
    x = np.ascontiguousarray(x, dtype=np.float32).astype(bf)
    z = np.ascontiguousarray(z, dtype=np.float32).astype(bf)
    # zt[p, kc*BJ + b*J + j] = z[b, j, kc*128 + p]
    zt = z.transpose(2, 0, 1).reshape(K, BJ)
    zt = np.concatenate([zt[0:128], zt[128:256]], axis=1)
    zt = np.ascontiguousarray(zt)
    in_maps = []
    for d in range(NCORES):
        xl = x[d * NL : (d + 1) * NL]                  # [a, i, K]
        xt = xl.transpose(2, 1, 0).reshape(K, AF)      # [K, (i, a)]
        xt = np.concatenate([xt[0:128], xt[128:256]], axis=1)
        in_maps.append({"xt": np.ascontiguousarray(xt), "zt": zt})
    return in_maps


def _epilogue(results):
    S = np.empty((N, N), dtype=np.float64)
    for d in range(NCORES):
        arr = results[d]["os"].astype(np.float32).reshape(128, NT, 1536)
        num = arr[:, :, 0:1024].reshape(128, NT, I, NL).sum(axis=2)    # [(b4,j), t, a]
        den = arr[:, :, 1024:1536].reshape(128, NT, 16, NL).sum(axis=2)
        r = num / den
        r = r.reshape(4, J, NT, NL).mean(axis=1).astype(np.float64)  # [b4, t, a]
        S[d * NL : (d + 1) * NL, :] = r.transpose(2, 1, 0).reshape(NL, N)
    diag = np.diagonal(S)
    m0 = S.max(axis=0)
    lx = m0 + np.log(np.exp(S - m0[None, :]).sum(axis=0)) - diag
    m1 = S.max(axis=1)
    lz = m1 + np.log(np.exp(S - m1[:, None]).sum(axis=1)) - diag
    loss = (lx + lz).mean()
    return np.asarray(loss, dtype=np.float32)


def run_on_device(x, z, trace=False):
    """Returns (loss, BassKernelResults)."""
    from concourse.bass_utils import run_bass_kernel_spmd

    global _cached
    if _cached is None:
        _cached = _build()
    nc = _cached
    in_maps = _prep_inputs(x, z)
    res = run_bass_kernel_spmd(nc, in_maps, list(range(NCORES)), trace=trace)
    return _epilogue(res.results), res


def kernel(x, z):
    loss, _ = run_on_device(x, z)
    return loss
